# revision 3
# baseline (speedup 1.0000x reference)
"""MultiHeadKANAttention Trainium2 kernel (8 NeuronCores, SPMD).

Token-sharded KANLinear + head-sharded attention, sim-tuned schedule:
  - phase 1: silu + 8 cubic B-spline basis planes (ACT/DVE/Pool hat
    recursion) feeding a K=9216 bf16 matmul chunk stream.
  - group kq [ke ko qe qo] (2048 cols, 8 PSUM banks); ACT frees the banks
    into fp16 SBUF copies; rope mults on DVE, combines on Pool; AllToAll
    wave 1 (1MB) re-shards k,q to 2 heads/core x all 2048 tokens.
  - group v (1024 cols) overlaps wave 1; single v wave (0.5MB).
  - kq unpack + 32 PE transposes after all KAN matmuls; collective-
    dependent loads are hosted on the Pool DMA queue so they cannot
    head-block the SP weight-stream queue.
  - attention per head-pair with transposed-scores layout: exp on ACT (no
    max subtraction needed: |scores| <= ~25), AV with an appended ones-row
    computing the softmax denominator for free; output projection trails
    one sq block behind its AV so the normalization chain is hidden.
  - partial out projection per core written as bf16; host sums 8 partials.

Verified vs the jax reference on HW: rel_l2 ~ 0.94e-2.
"""
import os
import numpy as np
import ml_dtypes

S = 2048
F = 1024
H = 16
HD = 64
O = 3 * F
CORES = 8
SLAB = S // CORES   # 256
NCH = 72            # 8 silu + 64 spline chunks
OKQ = 2048          # [ke ko qe qo] columns
BF16 = ml_dtypes.bfloat16

_PI = None


def _build_pi():
    """pi[new_row] = original qkv out_feature row. New order:
    [ke(512) | ko(512) | qe(512) | qo(512) | v_hp0_by_dest(512) | v_hp1_by_dest(512)]."""
    pi = np.zeros(O, dtype=np.int64)
    for h in range(H):
        base = h * 192
        for i in range(32):
            pi[0 * 512 + h * 32 + i] = base + 64 + 2 * i
            pi[1 * 512 + h * 32 + i] = base + 64 + 2 * i + 1
            pi[2 * 512 + h * 32 + i] = base + 2 * i
            pi[3 * 512 + h * 32 + i] = base + 2 * i + 1
    for hp in range(2):
        for c in range(CORES):
            h = 2 * c + hp
            for j in range(HD):
                pi[2048 + hp * 512 + c * 64 + j] = h * 192 + 128 + j
    return pi


def _host_prep(x, base_weight, spline_weight, spline_scaler, out_w, rot_cos, rot_sin):
    global _PI
    if _PI is None:
        _PI = _build_pi()
    pi = _PI
    x2 = np.asarray(x, np.float32).reshape(S, F)
    xT = np.ascontiguousarray(x2.T)

    W = np.asarray(base_weight, np.float32)[pi].copy()
    sw = np.asarray(spline_weight, np.float32) * np.asarray(spline_scaler, np.float32)[:, :, None]
    sw = sw[pi] * np.float32(1.0 / 6.0)
    W[0:1024] *= np.float32(0.125)      # fold 1/sqrt(HD) into k rows
    sw[0:1024] *= np.float32(0.125)

    wm0 = np.empty((NCH, 128, OKQ), np.float32)
    wmv = np.empty((NCH, 128, 1024), np.float32)
    for t in range(NCH):
        if t < 8:
            blk = W[:, t * 128:(t + 1) * 128]
        else:
            fb, c = (t - 8) // 8, (t - 8) % 8
            blk = sw[:, fb * 128:(fb + 1) * 128, c]
        wm0[t] = blk[0:2048].T
        wmv[t] = blk[2048:3072].T
    wm0 = wm0.astype(BF16)
    wmv = wmv.astype(BF16)

    Ct = np.tile(np.asarray(rot_cos, np.float32), (1, H))
    St = np.tile(np.asarray(rot_sin, np.float32), (1, H))
    wo_all = np.asarray(out_w, np.float32)

    in_maps = []
    for c in range(CORES):
        sl = slice(c * SLAB, (c + 1) * SLAB)
        in_maps.append({
            "xs": np.ascontiguousarray(xT[:, sl]),
            "cs": np.ascontiguousarray(Ct[sl]),
            "sn": np.ascontiguousarray(St[sl]),
            "wm0": wm0,
            "wmv": wmv,
            "wo": np.ascontiguousarray(wo_all[:, c * 128:(c + 1) * 128].T).astype(BF16),
        })
    return in_maps


def _build_program(single_core=False, reps=1):
    from contextlib import ExitStack
    import concourse.bass as bass
    import concourse.mybir as mybir
    import concourse.tile as tile
    from concourse import bacc
    from concourse.masks import make_identity

    dt = mybir.dt
    op = mybir.AluOpType
    AF = mybir.ActivationFunctionType
    PSUM = bass.MemorySpace.PSUM

    nc = bacc.Bacc("TRN2", target_bir_lowering=False, debug=False,
                   enable_asserts=False, num_devices=CORES)

    xs_d = nc.declare_dram_parameter("xs", [F, SLAB], dt.float32, isOutput=False)
    cs_d = nc.declare_dram_parameter("cs", [SLAB, 512], dt.float32, isOutput=False)
    sn_d = nc.declare_dram_parameter("sn", [SLAB, 512], dt.float32, isOutput=False)
    wm0_d = nc.declare_dram_parameter("wm0", [NCH, 128, OKQ], dt.bfloat16, isOutput=False)
    wmv_d = nc.declare_dram_parameter("wmv", [NCH, 128, 1024], dt.bfloat16, isOutput=False)
    wo_d = nc.declare_dram_parameter("wo", [128, F], dt.bfloat16, isOutput=False)
    out_d = nc.declare_dram_parameter("out", [S, F], dt.bfloat16, isOutput=True)
    if reps > 1:
        nc.declare_dram_parameter("reptag", [1, reps], dt.float32, isOutput=False)

    with tile.TileContext(nc, num_cores=CORES) as tc, ExitStack() as ctx:
        const = ctx.enter_context(tc.tile_pool(name="const", bufs=1))
        acts = ctx.enter_context(tc.tile_pool(name="acts", bufs=1))
        wstream = ctx.enter_context(tc.tile_pool(name="wstream", bufs=6))
        ropes = ctx.enter_context(tc.tile_pool(name="ropes", bufs=1))
        attn = ctx.enter_context(tc.tile_pool(name="attn", bufs=1))

        # ---- constants ----
        ident = const.tile([128, 128], dt.bfloat16, tag="ident", name="ident")
        make_identity(nc, ident[:])
        cs_sb = [const.tile([128, 512], dt.float32, tag=f"cs{tt}", name=f"cs{tt}") for tt in range(2)]
        sn_sb = [const.tile([128, 512], dt.float32, tag=f"sn{tt}", name=f"sn{tt}") for tt in range(2)]
        wo_sb = const.tile([128, F], dt.bfloat16, tag="wo", name="wo")
        xf_sb = [const.tile([128, SLAB], dt.float32, tag=f"xf{fb}", name=f"xf{fb}") for fb in range(8)]
        w0_first = const.tile([128, OKQ], dt.bfloat16, tag="w0f", name="w0f")
        nc.sync.dma_start(xf_sb[0][:], xs_d[0:128, :])
        nc.sync.dma_start(w0_first[:], wm0_d[0])
        for fb in range(1, 8):
            nc.sync.dma_start(xf_sb[fb][:], xs_d[fb * 128:(fb + 1) * 128, :])
        habias = {}
        for i in range(1, 11):
            bt = const.tile([128, 1], dt.float32, tag=f"bias{i}", name=f"bias{i}")
            nc.vector.memset(bt[:], float(5.5 - i))
            habias[i] = bt

        for rep in range(reps):
            a2a_kq_i = nc.dram_tensor(f"a2a_kq_i{rep}", [CORES, SLAB, 256], dt.bfloat16)
            a2a_kq_o = nc.dram_tensor(f"a2a_kq_o{rep}", [CORES, SLAB, 256], dt.bfloat16)
            a2a_v_i = nc.dram_tensor(f"a2a_v_i{rep}", [CORES, SLAB, 128], dt.bfloat16)
            a2a_v_o = nc.dram_tensor(f"a2a_v_o{rep}", [CORES, SLAB, 128], dt.bfloat16)

            # tmp pool lives only through phase 1 + rope; its space is
            # reused by attnbuf afterwards.
            tmp_cm = tc.tile_pool(name="tmp", bufs=2)
            tmp = tmp_cm.__enter__()
            # ---- phase 1: silu + b-spline basis chunks ----
            silu_sb = [acts.tile([128, SLAB], dt.bfloat16, tag=f"silu{fb}", name=f"silu{fb}")
                       for fb in range(8)]
            for fb in range(8):
                sg = tmp.tile([128, SLAB], dt.float32, tag="sg", name="sg")
                nc.scalar.activation(sg[:], xf_sb[fb][:], AF.Sigmoid)
                nc.vector.tensor_tensor(silu_sb[fb][:], xf_sb[fb][:], sg[:], op.mult)

            bs_sb = [[acts.tile([128, SLAB], dt.bfloat16, tag=f"bs{fb}_{c}", name=f"bs{fb}_{c}")
                      for c in range(8)] for fb in range(8)]
            for fb in range(8):
                xf = xf_sb[fb]
                u = tmp.tile([128, SLAB], dt.float32, tag="u", name="u", bufs=1)
                nc.vector.tensor_scalar(u[:], xf[:], 2.5, 5.5, op.mult, op.add)
                hats = []
                for i in range(1, 11):
                    z = tmp.tile([128, SLAB], dt.float32, tag="z", name="z", bufs=1)
                    nc.scalar.activation(z[:], xf[:], AF.Abs, bias=habias[i][:], scale=2.5)
                    hh = tmp.tile([128, SLAB], dt.float32, tag=f"h{i}", name=f"h{i}", bufs=1)
                    nc.scalar.activation(hh[:], z[:], AF.Relu, bias=1.0, scale=-1.0)
                    hats.append(hh)
                b2 = []
                for i in range(9):
                    ta = tmp.tile([128, SLAB], dt.float32, tag="ta", name="ta")
                    tb = tmp.tile([128, SLAB], dt.float32, tag="tb", name="tb")
                    nc.vector.scalar_tensor_tensor(ta[:], u[:], float(i), hats[i][:], op.subtract, op.mult)
                    nc.vector.scalar_tensor_tensor(tb[:], u[:], float(i + 3), hats[i + 1][:], op.subtract, op.mult)
                    bb = tmp.tile([128, SLAB], dt.float32, tag=f"b2_{i}", name=f"b2_{i}", bufs=1)
                    nc.gpsimd.tensor_tensor(bb[:], ta[:], tb[:], op.subtract)
                    b2.append(bb)
                for c in range(8):
                    ta = tmp.tile([128, SLAB], dt.float32, tag="ta", name="ta")
                    tb = tmp.tile([128, SLAB], dt.float32, tag="tb", name="tb")
                    nc.vector.scalar_tensor_tensor(ta[:], u[:], float(c), b2[c][:], op.subtract, op.mult)
                    nc.vector.scalar_tensor_tensor(tb[:], u[:], float(c + 4), b2[c + 1][:], op.subtract, op.mult)
                    nc.gpsimd.tensor_tensor(bs_sb[fb][c][:], ta[:], tb[:], op.subtract)

            def chunk_lhsT(t, tt):
                src = silu_sb[t] if t < 8 else bs_sb[(t - 8) // 8][(t - 8) % 8]
                return src[:, tt * 128:(tt + 1) * 128]

            # ---- group kq: [ke ko qe qo] matmul ----
            pack_kq = [ropes.tile([128, 8 * 256], dt.bfloat16, tag=f"pkq{tt}", name=f"pkq{tt}")
                       for tt in range(2)]
            pack_v = [ropes.tile([128, 8 * 128], dt.bfloat16, tag=f"pv{tt}", name=f"pv{tt}")
                      for tt in range(2)]

            psA_cm = tc.tile_pool(name="psA", bufs=1, space=PSUM)
            psA = psA_cm.__enter__()
            qp = [[psA.tile([128, 512], dt.float32, tag=f"qkv{tt}_{ot}", name=f"qkv{tt}_{ot}")
                   for ot in range(4)] for tt in range(2)]
            for t in range(NCH):
                if t == 0:
                    wt = w0_first
                else:
                    wt = wstream.tile([128, OKQ], dt.bfloat16, tag="w0", name="w0", bufs=4)
                    nc.sync.dma_start(wt[:], wm0_d[t])
                if t == 2 and rep == 0:
                    for tt in range(2):
                        nc.sync.dma_start(cs_sb[tt][:], cs_d[tt * 128:(tt + 1) * 128, :])
                        nc.sync.dma_start(sn_sb[tt][:], sn_d[tt * 128:(tt + 1) * 128, :])
                    nc.sync.dma_start(wo_sb[:], wo_d[:, :])
                for tt in range(2):
                    lhsT = chunk_lhsT(t, tt)
                    for ot in range(4):
                        nc.tensor.matmul(qp[tt][ot][:], lhsT, wt[:, ot * 512:(ot + 1) * 512],
                                         start=(t == 0), stop=(t == NCH - 1))

            # ACT frees the 8 kq PSUM banks fast (fp16 SBUF copies), then
            # rope: 4 mults on DVE, 2 combines on Pool per pair.
            kqc = [[ropes.tile([128, 512], dt.float16, tag=f"kqc{tt}{j}", name=f"kqc{tt}{j}")
                    for j in range(4)] for tt in range(2)]
            for tt in range(2):
                for j in range(4):
                    nc.scalar.activation(kqc[tt][j][:], qp[tt][j][:], AF.Copy)

            def rope_pair(tt, ea, oa, base, pack_t):
                """ea/oa: even/odd fp16 SBUF tiles -> rotated into pack_t;
                real part lands at dest_blk+base+hp*64+[0:32], imag at +[32:64]."""
                blk = pack_t[:].rearrange("p (d q) -> p d q", d=8)[:, :, base:base + 128]
                blk = blk.rearrange("p d (hp i) -> p d hp i", hp=2)
                t1 = tmp.tile([128, 512], dt.float32, tag="r1", name="r1", bufs=1)
                t2 = tmp.tile([128, 512], dt.float32, tag="r2", name="r2", bufs=1)
                nc.vector.tensor_tensor(t1[:], ea[:], cs_sb[tt][:], op.mult)
                nc.vector.tensor_tensor(t2[:], oa[:], sn_sb[tt][:], op.mult)
                nc.gpsimd.tensor_tensor(blk[:, :, :, 0:32], t1[:], t2[:], op.subtract)
                t3 = tmp.tile([128, 512], dt.float32, tag="r3", name="r3", bufs=1)
                t4 = tmp.tile([128, 512], dt.float32, tag="r4", name="r4", bufs=1)
                nc.vector.tensor_tensor(t3[:], ea[:], sn_sb[tt][:], op.mult)
                nc.vector.tensor_tensor(t4[:], oa[:], cs_sb[tt][:], op.mult)
                nc.gpsimd.tensor_tensor(blk[:, :, :, 32:64], t3[:], t4[:], op.add)

            rope_pair(0, kqc[0][0], kqc[0][1], 0, pack_kq[0])
            rope_pair(0, kqc[0][2], kqc[0][3], 128, pack_kq[0])
            nc.gpsimd.dma_start(
                a2a_kq_i.ap()[:, 0:128, :].rearrange("d p q -> p d q"),
                pack_kq[0][:].rearrange("p (d q) -> p d q", d=8))
            rope_pair(1, kqc[1][0], kqc[1][1], 0, pack_kq[1])
            rope_pair(1, kqc[1][2], kqc[1][3], 128, pack_kq[1])
            nc.gpsimd.dma_start(
                a2a_kq_i.ap()[:, 128:256, :].rearrange("d p q -> p d q"),
                pack_kq[1][:].rearrange("p (d q) -> p d q", d=8))
            if single_core:
                nc.gpsimd.dma_start(a2a_kq_o.ap(), a2a_kq_i.ap())
            else:
                nc.gpsimd.collective_compute(
                    "AllToAll", op.bypass, replica_groups=[list(range(CORES))],
                    ins=[a2a_kq_i.ap().opt()], outs=[a2a_kq_o.ap().opt()])

            tmp_cm.__exit__(None, None, None)

            # ---- group v (1024 cols: [hp0 by dest | hp1 by dest]) ----
            vq = [[psA.tile([128, 512], dt.float32, tag=f"qkv{tt}_{ot}", name=f"v{ot}_{tt}")
                   for ot in range(2)] for tt in range(2)]
            for t in range(NCH):
                wt = wstream.tile([128, 1024], dt.bfloat16, tag="wv", name="wv", bufs=5)
                nc.sync.dma_start(wt[:], wmv_d[t])
                for tt in range(2):
                    for ot in range(2):
                        nc.tensor.matmul(vq[tt][ot][:], chunk_lhsT(t, tt),
                                         wt[:, ot * 512:(ot + 1) * 512],
                                         start=(t == 0), stop=(t == NCH - 1))
            # pack: per tt interleave [hp0|hp1] per dest, then one wave
            for tt in range(2):
                w = pack_v[tt][:].rearrange("p (d x) -> p d x", d=8)
                for hp in range(2):
                    nc.vector.tensor_copy(
                        w[:, :, hp * 64:(hp + 1) * 64],
                        vq[tt][hp][:].rearrange("p (d j) -> p d j", d=8))
            # kqall first on the Pool queue: its wait (CC-kq) clears before
            # the pack-v DMAs' wait (DVE copies), so it must precede them.
            kqall = attn.tile([128, 16 * 256], dt.bfloat16, tag="kqall", name="kqall")
            nc.gpsimd.dma_start(
                kqall[:].rearrange("p (s a q) -> p s a q", s=8, a=2),
                a2a_kq_o.ap().rearrange("s (a p) q -> p s a q", a=2))
            for tt in range(2):
                nc.gpsimd.dma_start(
                    a2a_v_i.ap()[:, tt * 128:(tt + 1) * 128, :].rearrange("d p q -> p d q"),
                    pack_v[tt][:].rearrange("p (d q) -> p d q", d=8))
            if single_core:
                nc.gpsimd.dma_start(a2a_v_o.ap(), a2a_v_i.ap())
            else:
                nc.gpsimd.collective_compute(
                    "AllToAll", op.bypass, replica_groups=[list(range(CORES))],
                    ins=[a2a_v_i.ap().opt()], outs=[a2a_v_o.ap().opt()])
            vsall = attn.tile([128, 2 * 16 * 65], dt.bfloat16, tag="vst", name="vst")
            v5 = vsall[:].rearrange("p (hp s a j) -> p hp s a j", hp=2, s=8, a=2)
            nc.gpsimd.dma_start(
                v5[:, :, :, :, 0:64],
                a2a_v_o.ap().rearrange("s (a p) (hp j) -> p hp s a j", a=2, hp=2))
            nc.vector.memset(v5[:, :, :, :, 64:65], 1.0)
            vstat = [vsall[:, hp * 1040:(hp + 1) * 1040] for hp in range(2)]
            qT = attn.tile([128, S], dt.bfloat16, tag="qT", name="qT")
            kT = attn.tile([128, S], dt.bfloat16, tag="kT", name="kT")
            for idx in range(32):
                st = idx % 16
                is_q = idx >= 16
                tp = psA.tile([128, 128], dt.bfloat16,
                              tag=f"qkv{idx % 2}_{2 + (idx // 2) % 2}", name="tpk")
                nc.tensor.matmul(tp[:], kqall[:, st * 256 + (128 if is_q else 0):
                                              st * 256 + (256 if is_q else 128)],
                                 ident[:], is_transpose=True, skip_group_check=True)
                nc.vector.tensor_copy((qT if is_q else kT)[:, st * 128:(st + 1) * 128], tp[:])
            psA_cm.__exit__(None, None, None)

            # ---- attention: hp0 fully (its v lands first), then hp1 ----
            attnbuf_cm = tc.tile_pool(name="attnbuf", bufs=2)
            attnbuf = attnbuf_cm.__enter__()
            psB_cm = tc.tile_pool(name="psB", bufs=1, space=PSUM)
            psB = psB_cm.__enter__()
            ats = {}   # (hp, sq) -> list of 8 tiles
            ctx_sb = {}

            def emit_scores(hp, sq):
                hsl = slice(hp * 64, hp * 64 + 64)
                sqs = slice(sq * 512, (sq + 1) * 512)
                tiles = []
                for sm in range(8):
                    at = attnbuf.tile([128, 1024], dt.bfloat16, tag=f"at{hp}_{sm}",
                                      name=f"at{hp}_{sm}", bufs=2)
                    for half in range(2):
                        skc = 2 * sm + half
                        sc = psB.tile([128, 512], dt.float32, tag="sc", name="sc", bufs=3)
                        nc.tensor.matmul(sc[:],
                                         kT[hsl, skc * 128:(skc + 1) * 128],
                                         qT[hsl, sqs], start=True, stop=True)
                        nc.scalar.activation(at[:, half * 512:(half + 1) * 512], sc[:], AF.Exp)
                    tiles.append(at)
                ats[(hp, sq)] = tiles

            def emit_av(hp, sq):
                cx = psB.tile([65, 512], dt.float32, tag="cx", name="cx", bufs=2)
                for skc in range(16):
                    src_ap = ats[(hp, sq)][skc // 2][:, (skc % 2) * 512:(skc % 2 + 1) * 512]
                    nc.tensor.matmul(cx[:], vstat[hp][:, skc * 65:(skc + 1) * 65].opt(),
                                     src_ap, start=(skc == 0), stop=(skc == 15))
                rcp = attnbuf.tile([1, 512], dt.float32, tag="rcp", name="rcp")
                nc.vector.reciprocal(rcp[:], cx[64:65, :])
                rb = attnbuf.tile([64, 512], dt.float32, tag="rb", name="rb")
                nc.gpsimd.partition_broadcast(rb[:], rcp[:])
                if sq not in ctx_sb:
                    ctx_sb[sq] = attnbuf.tile([128, 512], dt.bfloat16, tag="ctx_sb",
                                              name="ctx_sb", bufs=4)
                hsl = slice(hp * 64, hp * 64 + 64)
                nc.vector.tensor_tensor(ctx_sb[sq][hsl, :], cx[0:64, :], rb[:], op.mult)

            def emit_outproj(sq):
                for tk in range(4):
                    for oh in range(2):
                        pr = psB.tile([128, 512], dt.float32, tag="pr", name="pr", bufs=3)
                        nc.tensor.matmul(pr[:], ctx_sb[sq][:, tk * 128:(tk + 1) * 128],
                                         wo_sb[:, oh * 512:(oh + 1) * 512], start=True, stop=True)
                        po = attnbuf.tile([128, 512], dt.bfloat16, tag="po", name="po", bufs=4)
                        nc.vector.tensor_copy(po[:], pr[:])
                        nc.sync.dma_start(out_d[sq * 512 + tk * 128: sq * 512 + (tk + 1) * 128,
                                                oh * 512:(oh + 1) * 512], po[:])
                del ctx_sb[sq]

            # 2-ahead score pipeline per head-pair; outproj trails its AV
            # pair by one sq so the normalization chain is hidden.
            for hp in range(2):
                emit_scores(hp, 0)
                emit_scores(hp, 1)
                for sq in range(4):
                    emit_av(hp, sq)
                    if sq + 2 < 4:
                        emit_scores(hp, sq + 2)
                    if hp == 1 and sq >= 1:
                        emit_outproj(sq - 1)
                if hp == 1:
                    emit_outproj(3)
            psB_cm.__exit__(None, None, None)
            attnbuf_cm.__exit__(None, None, None)

    nc.compile()
    return nc


_NC = None


def _get_program():
    global _NC
    if _NC is None:
        _NC = _build_program()
    return _NC


def kernel(**inputs):
    x = inputs["x"]
    out_b = np.asarray(inputs["out_b"], np.float32)
    in_maps = _host_prep(x, inputs["base_weight"], inputs["spline_weight"],
                         inputs["spline_scaler"], inputs["out_w"],
                         inputs["rot_cos"], inputs["rot_sin"])
    nc = _get_program()

    if os.environ.get("KAN_SIM"):
        results = _run_sim(nc, in_maps)
    else:
        from concourse.bass_utils import run_bass_kernel_spmd
        res = run_bass_kernel_spmd(nc, in_maps, core_ids=list(range(CORES)))
        kernel.last_results = res
        results = res.results

    out = np.zeros((S, F), np.float64)
    for c in range(CORES):
        out += np.asarray(results[c]["out"], np.float32)
    out = out.astype(np.float32) + out_b[None, :]
    return out.reshape(1, S, F)


def _run_sim(nc, in_maps):
    from concourse.bass_interp import MultiCoreSim
    sim = MultiCoreSim(nc, num_cores=CORES, num_workers=CORES)
    for c in range(CORES):
        core = sim.cores[c]
        for k, v in in_maps[c].items():
            core.tensor(k)[:] = v
    sim.simulate()
    return [{"out": np.array(sim.cores[c].tensor("out"))} for c in range(CORES)]


def make_timed_runner(in_maps=None, nc=None):
    """Device-resident jitted runner (mirrors bass2jax.run_bass_via_pjrt,
    no output donation) for repeat-timing the NEFF execution."""
    import time
    import jax
    import concourse.mybir as mybir
    from jax.sharding import Mesh, PartitionSpec, NamedSharding
    from jax.experimental.shard_map import shard_map
    from concourse import bass2jax

    nc = nc or _get_program()
    bass2jax.install_neuronx_cc_hook()
    partition_name = nc.partition_id_tensor.name if nc.partition_id_tensor else None
    in_names, out_names, out_avals, zero_outs = [], [], [], []
    for alloc in nc.m.functions[0].allocations:
        if not isinstance(alloc, mybir.MemoryLocationSet):
            continue
        name = alloc.memorylocations[0].name
        if alloc.kind == "ExternalInput":
            if name != partition_name:
                in_names.append(name)
        elif alloc.kind == "ExternalOutput":
            shape = tuple(alloc.tensor_shape)
            dtype = mybir.dt.np(alloc.dtype)
            out_names.append(name)
            out_avals.append(jax.core.ShapedArray(shape, dtype))
            zero_outs.append(np.zeros(shape, dtype))
    n_params = len(in_names)
    all_in = in_names + out_names
    if partition_name is not None:
        all_in.append(partition_name)

    def _body(*args):
        operands = list(args)
        if partition_name is not None:
            operands.append(bass2jax.partition_id_tensor())
        return tuple(bass2jax._bass_exec_p.bind(
            *operands, out_avals=tuple(out_avals), in_names=tuple(all_in),
            out_names=tuple(out_names), lowering_input_output_aliases=(),
            sim_require_finite=True, sim_require_nnan=True, nc=nc))

    devices = jax.devices()[:CORES]
    mesh = Mesh(np.asarray(devices), ("core",))
    nsh = NamedSharding(mesh, PartitionSpec("core"))
    sharded = jax.jit(shard_map(_body, mesh=mesh,
                                in_specs=(PartitionSpec("core"),) * (n_params + len(out_names)),
                                out_specs=(PartitionSpec("core"),) * len(out_names),
                                check_rep=False), keep_unused=True)
    concat_in = [np.concatenate([np.asarray(in_maps[c][k]) for c in range(CORES)], axis=0)
                 for k in in_names]
    concat_zero = [np.zeros((CORES * z.shape[0], *z.shape[1:]), z.dtype) for z in zero_outs]
    dev_args = [jax.device_put(a, nsh) for a in concat_in + concat_zero]

    def run_once():
        t0 = time.perf_counter()
        outs = sharded(*dev_args)
        jax.block_until_ready(outs)
        return time.perf_counter() - t0, outs

    return run_once, out_names, out_avals


# revision 4
# speedup vs baseline: 1.7134x; 1.7134x over previous
"""MultiHeadKANAttention Trainium2 kernel (8 NeuronCores, SPMD).

Token-sharded KANLinear + head-sharded attention, sim-tuned schedule:
  - phase 1: silu + 8 cubic B-spline basis planes (ACT/DVE/Pool hat
    recursion) feeding a K=9216 bf16 matmul chunk stream.
  - group kq [ke ko qe qo] (2048 cols, 8 PSUM banks); ACT frees the banks
    into fp16 SBUF copies; rope mults on DVE, combines on Pool; AllToAll
    wave 1 (1MB) re-shards k,q to 2 heads/core x all 2048 tokens.
  - group v (1024 cols) overlaps wave 1; single v wave (0.5MB).
  - kq unpack + 32 PE transposes after all KAN matmuls; collective-
    dependent loads are hosted on the Pool DMA queue so they cannot
    head-block the SP weight-stream queue.
  - attention per head-pair with transposed-scores layout: exp on ACT (no
    max subtraction needed: |scores| <= ~25), AV with an appended ones-row
    computing the softmax denominator for free; output projection trails
    one sq block behind its AV so the normalization chain is hidden.
  - partial out projection per core written as bf16; host sums 8 partials.

Verified vs the jax reference on HW: rel_l2 ~ 0.94e-2.
"""
import os
import numpy as np
import ml_dtypes

S = 2048
F = 1024
H = 16
HD = 64
O = 3 * F
CORES = 8
SLAB = S // CORES   # 256
NCH = 72            # 8 silu + 64 spline chunks
OKQ = 2048          # [ke ko qe qo] columns
BF16 = ml_dtypes.bfloat16

_PI = None


def _build_pi():
    """pi[new_row] = original qkv out_feature row. New order:
    [ke(512) | ko(512) | qe(512) | qo(512) | v_hp0_by_dest(512) | v_hp1_by_dest(512)]."""
    pi = np.zeros(O, dtype=np.int64)
    for h in range(H):
        base = h * 192
        for i in range(32):
            pi[0 * 512 + h * 32 + i] = base + 64 + 2 * i
            pi[1 * 512 + h * 32 + i] = base + 64 + 2 * i + 1
            pi[2 * 512 + h * 32 + i] = base + 2 * i
            pi[3 * 512 + h * 32 + i] = base + 2 * i + 1
    for hp in range(2):
        for c in range(CORES):
            h = 2 * c + hp
            for j in range(HD):
                pi[2048 + hp * 512 + c * 64 + j] = h * 192 + 128 + j
    return pi


def _host_prep(x, base_weight, spline_weight, spline_scaler, out_w, rot_cos, rot_sin):
    global _PI
    if _PI is None:
        _PI = _build_pi()
    pi = _PI
    x2 = np.asarray(x, np.float32).reshape(S, F)
    xT = np.ascontiguousarray(x2.T)

    W = np.asarray(base_weight, np.float32)[pi].copy()
    sw = np.asarray(spline_weight, np.float32) * np.asarray(spline_scaler, np.float32)[:, :, None]
    sw = sw[pi] * np.float32(1.0 / 6.0)
    W[0:1024] *= np.float32(0.125)      # fold 1/sqrt(HD) into k rows
    sw[0:1024] *= np.float32(0.125)

    wm0 = np.empty((NCH, 128, OKQ), np.float32)
    wmv = np.empty((NCH, 128, 1024), np.float32)
    for t in range(NCH):
        if t < 8:
            blk = W[:, t * 128:(t + 1) * 128]
        else:
            fb, c = (t - 8) // 8, (t - 8) % 8
            blk = sw[:, fb * 128:(fb + 1) * 128, c]
        wm0[t] = blk[0:2048].T
        wmv[t] = blk[2048:3072].T
    wm0 = wm0.astype(BF16)
    wmv = wmv.astype(BF16)

    Ct = np.tile(np.asarray(rot_cos, np.float32), (1, H))
    St = np.tile(np.asarray(rot_sin, np.float32), (1, H))
    wo_all = np.asarray(out_w, np.float32)

    in_maps = []
    for c in range(CORES):
        sl = slice(c * SLAB, (c + 1) * SLAB)
        in_maps.append({
            "xs": np.ascontiguousarray(xT[:, sl]),
            "cs": np.ascontiguousarray(Ct[sl]),
            "sn": np.ascontiguousarray(St[sl]),
            "wm0": wm0,
            "wmv": wmv,
            "wo": np.ascontiguousarray(wo_all[:, c * 128:(c + 1) * 128].T).astype(BF16),
        })
    return in_maps


def _build_program(single_core=False, reps=1):
    from contextlib import ExitStack
    import concourse.bass as bass
    import concourse.mybir as mybir
    import concourse.tile as tile
    from concourse import bacc
    from concourse.masks import make_identity

    dt = mybir.dt
    op = mybir.AluOpType
    AF = mybir.ActivationFunctionType
    PSUM = bass.MemorySpace.PSUM

    nc = bacc.Bacc("TRN2", target_bir_lowering=False, debug=False,
                   enable_asserts=False, num_devices=CORES)

    xs_d = nc.declare_dram_parameter("xs", [F, SLAB], dt.float32, isOutput=False)
    cs_d = nc.declare_dram_parameter("cs", [SLAB, 512], dt.float32, isOutput=False)
    sn_d = nc.declare_dram_parameter("sn", [SLAB, 512], dt.float32, isOutput=False)
    wm0_d = nc.declare_dram_parameter("wm0", [NCH, 128, OKQ], dt.bfloat16, isOutput=False)
    wmv_d = nc.declare_dram_parameter("wmv", [NCH, 128, 1024], dt.bfloat16, isOutput=False)
    wo_d = nc.declare_dram_parameter("wo", [128, F], dt.bfloat16, isOutput=False)
    out_d = nc.declare_dram_parameter("out", [S, F], dt.bfloat16, isOutput=True)
    if reps > 1:
        nc.declare_dram_parameter("reptag", [1, reps], dt.float32, isOutput=False)

    with tile.TileContext(nc, num_cores=CORES) as tc, ExitStack() as ctx:
        const = ctx.enter_context(tc.tile_pool(name="const", bufs=1))
        acts = ctx.enter_context(tc.tile_pool(name="acts", bufs=1))
        wstream = ctx.enter_context(tc.tile_pool(name="wstream", bufs=6))
        ropes = ctx.enter_context(tc.tile_pool(name="ropes", bufs=1))
        attn = ctx.enter_context(tc.tile_pool(name="attn", bufs=1))

        # ---- constants ----
        ident = const.tile([128, 128], dt.bfloat16, tag="ident", name="ident")
        make_identity(nc, ident[:])
        cs_sb = [const.tile([128, 512], dt.float32, tag=f"cs{tt}", name=f"cs{tt}") for tt in range(2)]
        sn_sb = [const.tile([128, 512], dt.float32, tag=f"sn{tt}", name=f"sn{tt}") for tt in range(2)]
        wo_sb = const.tile([128, F], dt.bfloat16, tag="wo", name="wo")
        xf_sb = [const.tile([128, SLAB], dt.float32, tag=f"xf{fb}", name=f"xf{fb}") for fb in range(8)]
        w0_first = const.tile([128, OKQ], dt.bfloat16, tag="w0f", name="w0f")
        nc.sync.dma_start(xf_sb[0][:], xs_d[0:128, :])
        nc.sync.dma_start(w0_first[:], wm0_d[0])
        for fb in range(1, 8):
            nc.sync.dma_start(xf_sb[fb][:], xs_d[fb * 128:(fb + 1) * 128, :])
        habias = {}
        for i in range(1, 11):
            bt = const.tile([128, 1], dt.float32, tag=f"bias{i}", name=f"bias{i}")
            nc.vector.memset(bt[:], float(5.5 - i))
            habias[i] = bt

        for rep in range(reps):
            a2a_kq_i = nc.dram_tensor(f"a2a_kq_i{rep}", [CORES, SLAB, 256], dt.bfloat16)
            a2a_kq_o = nc.dram_tensor(f"a2a_kq_o{rep}", [CORES, SLAB, 256], dt.bfloat16)
            a2a_v_i = nc.dram_tensor(f"a2a_v_i{rep}", [CORES, SLAB, 128], dt.bfloat16)
            a2a_v_o = nc.dram_tensor(f"a2a_v_o{rep}", [CORES, SLAB, 128], dt.bfloat16)

            # tmp pool lives only through phase 1 + rope; its space is
            # reused by attnbuf afterwards.
            tmp_cm = tc.tile_pool(name="tmp", bufs=2)
            tmp = tmp_cm.__enter__()
            # ---- phase 1: silu + b-spline basis chunks ----
            silu_sb = [acts.tile([128, SLAB], dt.bfloat16, tag=f"silu{fb}", name=f"silu{fb}")
                       for fb in range(8)]
            for fb in range(8):
                sg = tmp.tile([128, SLAB], dt.float32, tag="sg", name="sg")
                nc.scalar.activation(sg[:], xf_sb[fb][:], AF.Sigmoid)
                nc.vector.tensor_tensor(silu_sb[fb][:], xf_sb[fb][:], sg[:], op.mult)

            bs_sb = [[acts.tile([128, SLAB], dt.bfloat16, tag=f"bs{fb}_{c}", name=f"bs{fb}_{c}")
                      for c in range(8)] for fb in range(8)]
            for fb in range(8):
                xf = xf_sb[fb]
                u = tmp.tile([128, SLAB], dt.float32, tag="u", name="u", bufs=1)
                nc.vector.tensor_scalar(u[:], xf[:], 2.5, 5.5, op.mult, op.add)
                hats = []
                for i in range(1, 11):
                    z = tmp.tile([128, SLAB], dt.float32, tag="z", name="z", bufs=1)
                    nc.scalar.activation(z[:], xf[:], AF.Abs, bias=habias[i][:], scale=2.5)
                    hh = tmp.tile([128, SLAB], dt.float32, tag=f"h{i}", name=f"h{i}", bufs=1)
                    nc.scalar.activation(hh[:], z[:], AF.Relu, bias=1.0, scale=-1.0)
                    hats.append(hh)
                b2 = []
                for i in range(9):
                    ta = tmp.tile([128, SLAB], dt.float32, tag="ta", name="ta")
                    tb = tmp.tile([128, SLAB], dt.float32, tag="tb", name="tb")
                    nc.vector.scalar_tensor_tensor(ta[:], u[:], float(i), hats[i][:], op.subtract, op.mult)
                    nc.vector.scalar_tensor_tensor(tb[:], u[:], float(i + 3), hats[i + 1][:], op.subtract, op.mult)
                    bb = tmp.tile([128, SLAB], dt.float32, tag=f"b2_{i}", name=f"b2_{i}", bufs=1)
                    nc.gpsimd.tensor_tensor(bb[:], ta[:], tb[:], op.subtract)
                    b2.append(bb)
                for c in range(8):
                    ta = tmp.tile([128, SLAB], dt.float32, tag="ta", name="ta")
                    tb = tmp.tile([128, SLAB], dt.float32, tag="tb", name="tb")
                    nc.vector.scalar_tensor_tensor(ta[:], u[:], float(c), b2[c][:], op.subtract, op.mult)
                    nc.vector.scalar_tensor_tensor(tb[:], u[:], float(c + 4), b2[c + 1][:], op.subtract, op.mult)
                    nc.gpsimd.tensor_tensor(bs_sb[fb][c][:], ta[:], tb[:], op.subtract)

            def chunk_lhsT(t, tt):
                src = silu_sb[t] if t < 8 else bs_sb[(t - 8) // 8][(t - 8) % 8]
                return src[:, tt * 128:(tt + 1) * 128]

            # ---- group kq: [ke ko qe qo] matmul ----
            pack_kq = [ropes.tile([128, 8 * 256], dt.bfloat16, tag=f"pkq{tt}", name=f"pkq{tt}")
                       for tt in range(2)]
            pack_v = [ropes.tile([128, 8 * 128], dt.bfloat16, tag=f"pv{tt}", name=f"pv{tt}")
                      for tt in range(2)]

            psA_cm = tc.tile_pool(name="psA", bufs=1, space=PSUM)
            psA = psA_cm.__enter__()
            qp = [[psA.tile([128, 512], dt.float32, tag=f"qkv{tt}_{ot}", name=f"qkv{tt}_{ot}")
                   for ot in range(4)] for tt in range(2)]
            for t in range(NCH):
                if t == 0:
                    wt = w0_first
                else:
                    wt = wstream.tile([128, OKQ], dt.bfloat16, tag="w0", name="w0", bufs=4)
                    nc.sync.dma_start(wt[:], wm0_d[t])
                if t == 2 and rep == 0:
                    for tt in range(2):
                        nc.sync.dma_start(cs_sb[tt][:], cs_d[tt * 128:(tt + 1) * 128, :])
                        nc.sync.dma_start(sn_sb[tt][:], sn_d[tt * 128:(tt + 1) * 128, :])
                    nc.sync.dma_start(wo_sb[:], wo_d[:, :])
                for tt in range(2):
                    lhsT = chunk_lhsT(t, tt)
                    for ot in range(4):
                        nc.tensor.matmul(qp[tt][ot][:], lhsT, wt[:, ot * 512:(ot + 1) * 512],
                                         start=(t == 0), stop=(t == NCH - 1))

            # ACT frees the 8 kq PSUM banks fast (fp16 SBUF copies), then
            # rope: 4 mults on DVE, 2 combines on Pool per pair.
            kqc = [[ropes.tile([128, 512], dt.float16, tag=f"kqc{tt}{j}", name=f"kqc{tt}{j}")
                    for j in range(4)] for tt in range(2)]
            for tt in range(2):
                for j in range(4):
                    nc.scalar.activation(kqc[tt][j][:], qp[tt][j][:], AF.Copy)

            def rope_pair(tt, ea, oa, base, pack_t):
                """ea/oa: even/odd fp16 SBUF tiles -> rotated into pack_t;
                real part lands at dest_blk+base+hp*64+[0:32], imag at +[32:64]."""
                blk = pack_t[:].rearrange("p (d q) -> p d q", d=8)[:, :, base:base + 128]
                blk = blk.rearrange("p d (hp i) -> p d hp i", hp=2)
                t1 = tmp.tile([128, 512], dt.float32, tag="r1", name="r1", bufs=1)
                t2 = tmp.tile([128, 512], dt.float32, tag="r2", name="r2", bufs=1)
                nc.vector.tensor_tensor(t1[:], ea[:], cs_sb[tt][:], op.mult)
                nc.vector.tensor_tensor(t2[:], oa[:], sn_sb[tt][:], op.mult)
                nc.gpsimd.tensor_tensor(blk[:, :, :, 0:32], t1[:], t2[:], op.subtract)
                t3 = tmp.tile([128, 512], dt.float32, tag="r3", name="r3", bufs=1)
                t4 = tmp.tile([128, 512], dt.float32, tag="r4", name="r4", bufs=1)
                nc.vector.tensor_tensor(t3[:], ea[:], sn_sb[tt][:], op.mult)
                nc.vector.tensor_tensor(t4[:], oa[:], cs_sb[tt][:], op.mult)
                nc.gpsimd.tensor_tensor(blk[:, :, :, 32:64], t3[:], t4[:], op.add)

            rope_pair(0, kqc[0][0], kqc[0][1], 0, pack_kq[0])
            rope_pair(0, kqc[0][2], kqc[0][3], 128, pack_kq[0])
            nc.gpsimd.dma_start(
                a2a_kq_i.ap()[:, 0:128, :].rearrange("d p q -> p d q"),
                pack_kq[0][:].rearrange("p (d q) -> p d q", d=8))
            rope_pair(1, kqc[1][0], kqc[1][1], 0, pack_kq[1])
            rope_pair(1, kqc[1][2], kqc[1][3], 128, pack_kq[1])
            nc.gpsimd.dma_start(
                a2a_kq_i.ap()[:, 128:256, :].rearrange("d p q -> p d q"),
                pack_kq[1][:].rearrange("p (d q) -> p d q", d=8))
            if single_core:
                nc.gpsimd.dma_start(a2a_kq_o.ap(), a2a_kq_i.ap())
            else:
                nc.gpsimd.collective_compute(
                    "AllToAll", op.bypass, replica_groups=[list(range(CORES))],
                    ins=[a2a_kq_i.ap().opt()], outs=[a2a_kq_o.ap().opt()])

            tmp_cm.__exit__(None, None, None)

            # ---- group v (1024 cols: [hp0 by dest | hp1 by dest]) ----
            vq = [[psA.tile([128, 512], dt.float32, tag=f"qkv{tt}_{ot}", name=f"v{ot}_{tt}")
                   for ot in range(2)] for tt in range(2)]
            for t in range(NCH):
                wt = wstream.tile([128, 1024], dt.bfloat16, tag="wv", name="wv", bufs=6)
                nc.sync.dma_start(wt[:], wmv_d[t])
                for tt in range(2):
                    for ot in range(2):
                        nc.tensor.matmul(vq[tt][ot][:], chunk_lhsT(t, tt),
                                         wt[:, ot * 512:(ot + 1) * 512],
                                         start=(t == 0), stop=(t == NCH - 1))
            # pack: per tt interleave [hp0|hp1] per dest, then one wave
            for tt in range(2):
                w = pack_v[tt][:].rearrange("p (d x) -> p d x", d=8)
                for hp in range(2):
                    nc.vector.tensor_copy(
                        w[:, :, hp * 64:(hp + 1) * 64],
                        vq[tt][hp][:].rearrange("p (d j) -> p d j", d=8))
            # kqall first on the Pool queue: its wait (CC-kq) clears before
            # the pack-v DMAs' wait (DVE copies), so it must precede them.
            kqall = attn.tile([128, 16 * 256], dt.bfloat16, tag="kqall", name="kqall")
            nc.gpsimd.dma_start(
                kqall[:].rearrange("p (s a q) -> p s a q", s=8, a=2),
                a2a_kq_o.ap().rearrange("s (a p) q -> p s a q", a=2))
            for tt in range(2):
                nc.gpsimd.dma_start(
                    a2a_v_i.ap()[:, tt * 128:(tt + 1) * 128, :].rearrange("d p q -> p d q"),
                    pack_v[tt][:].rearrange("p (d q) -> p d q", d=8))
            if single_core:
                nc.gpsimd.dma_start(a2a_v_o.ap(), a2a_v_i.ap())
            else:
                nc.gpsimd.collective_compute(
                    "AllToAll", op.bypass, replica_groups=[list(range(CORES))],
                    ins=[a2a_v_i.ap().opt()], outs=[a2a_v_o.ap().opt()])
            vsall = attn.tile([128, 2 * 16 * 65], dt.bfloat16, tag="vst", name="vst")
            v5 = vsall[:].rearrange("p (hp s a j) -> p hp s a j", hp=2, s=8, a=2)
            nc.gpsimd.dma_start(
                v5[:, :, :, :, 0:64],
                a2a_v_o.ap().rearrange("s (a p) (hp j) -> p hp s a j", a=2, hp=2))
            nc.vector.memset(v5[:, :, :, :, 64:65], 1.0)
            vstat = [vsall[:, hp * 1040:(hp + 1) * 1040] for hp in range(2)]
            qT = attn.tile([128, S], dt.bfloat16, tag="qT", name="qT")
            kT = attn.tile([128, S], dt.bfloat16, tag="kT", name="kT")
            # order: q st0-3 (needed by the first score block), then all k
            # tiles in skc consumption order, then the remaining q tiles.
            t_order = ([(st, True) for st in range(4)] +
                       [(st, False) for st in range(16)] +
                       [(st, True) for st in range(4, 16)])
            for idx, (st, is_q) in enumerate(t_order):
                tp = psA.tile([128, 128], dt.bfloat16,
                              tag=f"qkv{idx % 2}_{2 + (idx // 2) % 2}", name="tpk")
                nc.tensor.matmul(tp[:], kqall[:, st * 256 + (128 if is_q else 0):
                                              st * 256 + (256 if is_q else 128)],
                                 ident[:], is_transpose=True, skip_group_check=True)
                nc.vector.tensor_copy((qT if is_q else kT)[:, st * 128:(st + 1) * 128], tp[:])
            psA_cm.__exit__(None, None, None)

            # ---- attention: hp0 fully (its v lands first), then hp1 ----
            attnbuf_cm = tc.tile_pool(name="attnbuf", bufs=2)
            attnbuf = attnbuf_cm.__enter__()
            psB_cm = tc.tile_pool(name="psB", bufs=1, space=PSUM)
            psB = psB_cm.__enter__()
            ats = {}   # (hp, sq) -> list of 8 tiles
            ctx_sb = {}

            def emit_scores(hp, sq):
                hsl = slice(hp * 64, hp * 64 + 64)
                sqs = slice(sq * 512, (sq + 1) * 512)
                tiles = []
                for sm in range(8):
                    at = attnbuf.tile([128, 1024], dt.bfloat16, tag=f"at{hp}_{sm}",
                                      name=f"at{hp}_{sm}", bufs=2)
                    for half in range(2):
                        skc = 2 * sm + half
                        sc = psB.tile([128, 512], dt.float32, tag="sc", name="sc", bufs=3)
                        nc.tensor.matmul(sc[:],
                                         kT[hsl, skc * 128:(skc + 1) * 128],
                                         qT[hsl, sqs], start=True, stop=True)
                        nc.scalar.activation(at[:, half * 512:(half + 1) * 512], sc[:], AF.Exp)
                    tiles.append(at)
                ats[(hp, sq)] = tiles

            def emit_av(hp, sq):
                cx = psB.tile([65, 512], dt.float32, tag="cx", name="cx", bufs=2)
                for skc in range(16):
                    src_ap = ats[(hp, sq)][skc // 2][:, (skc % 2) * 512:(skc % 2 + 1) * 512]
                    nc.tensor.matmul(cx[:], vstat[hp][:, skc * 65:(skc + 1) * 65].opt(),
                                     src_ap, start=(skc == 0), stop=(skc == 15))
                rcp = attnbuf.tile([1, 512], dt.float32, tag="rcp", name="rcp")
                nc.vector.reciprocal(rcp[:], cx[64:65, :])
                rb = attnbuf.tile([64, 512], dt.float32, tag="rb", name="rb")
                nc.gpsimd.partition_broadcast(rb[:], rcp[:])
                if sq not in ctx_sb:
                    ctx_sb[sq] = attnbuf.tile([128, 512], dt.bfloat16, tag="ctx_sb",
                                              name="ctx_sb", bufs=4)
                hsl = slice(hp * 64, hp * 64 + 64)
                nc.vector.tensor_tensor(ctx_sb[sq][hsl, :], cx[0:64, :], rb[:], op.mult)

            def emit_outproj(sq):
                for tk in range(4):
                    for oh in range(2):
                        pr = psB.tile([128, 512], dt.float32, tag="pr", name="pr", bufs=3)
                        nc.tensor.matmul(pr[:], ctx_sb[sq][:, tk * 128:(tk + 1) * 128],
                                         wo_sb[:, oh * 512:(oh + 1) * 512], start=True, stop=True)
                        po = attnbuf.tile([128, 512], dt.bfloat16, tag="po", name="po", bufs=4)
                        nc.vector.tensor_copy(po[:], pr[:])
                        nc.sync.dma_start(out_d[sq * 512 + tk * 128: sq * 512 + (tk + 1) * 128,
                                                oh * 512:(oh + 1) * 512], po[:])
                del ctx_sb[sq]

            # 2-ahead score pipeline per head-pair; outproj trails its AV
            # pair by one sq so the normalization chain is hidden.
            for hp in range(2):
                emit_scores(hp, 0)
                emit_scores(hp, 1)
                for sq in range(4):
                    emit_av(hp, sq)
                    if sq + 2 < 4:
                        emit_scores(hp, sq + 2)
                    if hp == 1 and sq >= 1:
                        emit_outproj(sq - 1)
                if hp == 1:
                    emit_outproj(3)
            psB_cm.__exit__(None, None, None)
            attnbuf_cm.__exit__(None, None, None)

    nc.compile()
    return nc


_NC = None


def _get_program():
    global _NC
    if _NC is None:
        _NC = _build_program()
    return _NC


def kernel(**inputs):
    x = inputs["x"]
    out_b = np.asarray(inputs["out_b"], np.float32)
    in_maps = _host_prep(x, inputs["base_weight"], inputs["spline_weight"],
                         inputs["spline_scaler"], inputs["out_w"],
                         inputs["rot_cos"], inputs["rot_sin"])
    nc = _get_program()

    if os.environ.get("KAN_SIM"):
        results = _run_sim(nc, in_maps)
    else:
        from concourse.bass_utils import run_bass_kernel_spmd
        res = run_bass_kernel_spmd(nc, in_maps, core_ids=list(range(CORES)))
        kernel.last_results = res
        results = res.results

    out = np.zeros((S, F), np.float64)
    for c in range(CORES):
        out += np.asarray(results[c]["out"], np.float32)
    out = out.astype(np.float32) + out_b[None, :]
    return out.reshape(1, S, F)


def _run_sim(nc, in_maps):
    from concourse.bass_interp import MultiCoreSim
    sim = MultiCoreSim(nc, num_cores=CORES, num_workers=CORES)
    for c in range(CORES):
        core = sim.cores[c]
        for k, v in in_maps[c].items():
            core.tensor(k)[:] = v
    sim.simulate()
    return [{"out": np.array(sim.cores[c].tensor("out"))} for c in range(CORES)]


def make_timed_runner(in_maps=None, nc=None):
    """Device-resident jitted runner (mirrors bass2jax.run_bass_via_pjrt,
    no output donation) for repeat-timing the NEFF execution."""
    import time
    import jax
    import concourse.mybir as mybir
    from jax.sharding import Mesh, PartitionSpec, NamedSharding
    from jax.experimental.shard_map import shard_map
    from concourse import bass2jax

    nc = nc or _get_program()
    bass2jax.install_neuronx_cc_hook()
    partition_name = nc.partition_id_tensor.name if nc.partition_id_tensor else None
    in_names, out_names, out_avals, zero_outs = [], [], [], []
    for alloc in nc.m.functions[0].allocations:
        if not isinstance(alloc, mybir.MemoryLocationSet):
            continue
        name = alloc.memorylocations[0].name
        if alloc.kind == "ExternalInput":
            if name != partition_name:
                in_names.append(name)
        elif alloc.kind == "ExternalOutput":
            shape = tuple(alloc.tensor_shape)
            dtype = mybir.dt.np(alloc.dtype)
            out_names.append(name)
            out_avals.append(jax.core.ShapedArray(shape, dtype))
            zero_outs.append(np.zeros(shape, dtype))
    n_params = len(in_names)
    all_in = in_names + out_names
    if partition_name is not None:
        all_in.append(partition_name)

    def _body(*args):
        operands = list(args)
        if partition_name is not None:
            operands.append(bass2jax.partition_id_tensor())
        return tuple(bass2jax._bass_exec_p.bind(
            *operands, out_avals=tuple(out_avals), in_names=tuple(all_in),
            out_names=tuple(out_names), lowering_input_output_aliases=(),
            sim_require_finite=True, sim_require_nnan=True, nc=nc))

    devices = jax.devices()[:CORES]
    mesh = Mesh(np.asarray(devices), ("core",))
    nsh = NamedSharding(mesh, PartitionSpec("core"))
    sharded = jax.jit(shard_map(_body, mesh=mesh,
                                in_specs=(PartitionSpec("core"),) * (n_params + len(out_names)),
                                out_specs=(PartitionSpec("core"),) * len(out_names),
                                check_rep=False), keep_unused=True)
    concat_in = [np.concatenate([np.asarray(in_maps[c][k]) for c in range(CORES)], axis=0)
                 for k in in_names]
    concat_zero = [np.zeros((CORES * z.shape[0], *z.shape[1:]), z.dtype) for z in zero_outs]
    dev_args = [jax.device_put(a, nsh) for a in concat_in + concat_zero]

    def run_once():
        t0 = time.perf_counter()
        outs = sharded(*dev_args)
        jax.block_until_ready(outs)
        return time.perf_counter() - t0, outs

    return run_once, out_names, out_avals
